# revision 66
# baseline (speedup 1.0000x reference)
"""Trainium2 Bass kernel for nn_Cell_67894843015282 (DARTS-style NAS cell).

Strategy (v2, fp8 DoubleRow):
  - All routing/gating logic computed on host (jax-on-CPU f32 to match the
    reference step() comparisons bit-for-bit).
  - BN affine + channel gates + op coefficients folded into fused per-tap
    dense matrices M_t[c,o] = dw[c,t] * pw_scaled[o,c]; depthwise+pointwise
    conv = sum over taps of M_t^T @ shifted_window(x).
  - All taps run on the tensor engine as fp8e4m3 DoubleRow matmuls: two taps
    share one matmul (K=256) via a strided k-tile access pattern on a padded
    fp8 image. Constraint (hw): the k-tile stride must be a multiple of 4, so
    taps pair when their dx differ by 0 mod 4 (pad widths are multiples of 4).
    This gives ~4x tensor-engine throughput vs per-tap fp32r matmuls.
  - Power-of-2 scale chains (per-state SX, per-sep-mid SM, per-step SACC)
    keep fp8 operands in range; scales calibrated from a host numpy run.
    PSUM accumulates in f32; the scale is divided out at the state merge.
  - Pools (max/avg 3x3) on the vector engine in bf16 (2x DVE mode) from
    bf16 state copies; skip/pool contributions accumulate into an f32
    SBUF `extra` tensor (DVE + some on gpsimd); states stay f32.
  - Data parallel over batch: 1 image per NeuronCore, 8 cores.
"""

import os

import numpy as np

B, C, HH, WW = 8, 128, 32, 32
PIX = HH * WW
C_PREV = 512
STEPS, N_EDGES, N_OPS = 4, 14, 8
N_CORES = 8

RPAD_P, RPAD_W = 4, 40  # padded relu image: [128, 40, 40]
MPAD_P, MPAD_W = 2, 36  # padded sep-mid image: [128, 36, 36]

ACT_TARGET = 64.0   # target absmax of fp8-scaled activations (e4m3 max 240)
W_TARGET = 96.0     # target absmax of fp8-scaled weights

# ---------------------------------------------------------------------------
# Host-side gating / fusion (the "plan")
# ---------------------------------------------------------------------------


def _f32(x):
    return np.asarray(x, dtype=np.float32)


def _fp8_dtype():
    import ml_dtypes

    return ml_dtypes.float8_e4m3


def _gate_math(inputs):
    """Replicate the data-independent gating chain of the reference in f32."""
    try:
        import jax

        cpu = jax.devices("cpu")[0]

        with jax.default_device(cpu):
            import jax.numpy as jnp

            return _gate_math_impl(jnp, jax.nn.sigmoid, inputs, to_np=np.asarray)
    except Exception:

        def np_sig(x):
            return 1.0 / (1.0 + np.exp(-np.asarray(x, np.float32), dtype=np.float32))

        return _gate_math_impl(np, np_sig, inputs, to_np=np.asarray)


def _gate_math_impl(xp, sig, inputs, to_np):
    f32 = np.float32
    weights2 = xp.asarray(inputs["weights2"], dtype=f32)
    thre = xp.asarray(inputs["thre"], dtype=f32)
    mask_default = xp.asarray(inputs["mask_default"])
    kernel_param = xp.asarray(inputs["kernel_param"], dtype=f32)
    mask_k_default = xp.asarray(inputs["mask_k_default"])
    mask_w_default = xp.asarray(inputs["mask_w_default"])
    kernel_pre = xp.asarray(inputs["kernel_pre"], dtype=f32)
    thre_pre = xp.asarray(inputs["thre_pre"], dtype=f32)

    def step(x):
        return (x > 0).astype(f32)

    mdf = mask_default.astype(f32)

    g0 = sig(kernel_pre[0])
    mk0 = step(g0 - thre_pre[0])
    gv0 = to_np(g0 * mk0).astype(f32)
    g1p = sig(kernel_pre[1])
    mk1 = step(g1p - thre_pre[1])
    gv1 = to_np(g1p * mk1).astype(f32)

    n_states = 2
    offset = 0
    m_all = np.zeros((N_EDGES, N_OPS), np.float32)
    for i in range(STEPS):
        n = n_states
        weight_sum = (weights2[offset : offset + n] * mdf[offset : offset + n]).sum()
        for j in range(n):
            e = offset + j
            ns = weight_sum
            m_list = []
            for k in range(N_OPS):
                w = weights2[e, k]
                md = mdf[e, k]
                m = xp.where(
                    md == 0, f32(0.0), xp.where(w != ns, step(w - thre[e, k, 0]), md)
                )
                cond = (md != 0) & (w != ns) & (m == 0)
                m_list.append(m)
                ns = xp.where(cond, ns - w, ns)
            m_vec = xp.stack(m_list)
            weight_sum = (
                weight_sum - (weights2[e] * mdf[e]).sum() + (weights2[e] * m_vec).sum()
            )
            m_all[e] = to_np(m_vec)
        offset += n
        n_states += 1

    coef = to_np(weights2).astype(f32) * m_all

    gates = to_np(sig(kernel_param)).astype(f32)
    t1 = to_np(thre[:, :, 1]).astype(f32)
    t2 = to_np(thre[:, :, 2]).astype(f32)
    mk = (gates - t1[:, :, None] > 0).astype(f32) * (to_np(mask_k_default) != 0)
    mw = (gates - t2[:, :, None] > 0).astype(f32) * (to_np(mask_w_default) != 0)
    return dict(
        gv0=gv0,
        gv1=gv1,
        coef=coef,
        gates=gates,
        mk=mk.astype(np.float32),
        mw=mw.astype(np.float32),
    )


TAPS3 = [(dy, dx) for dy in (-1, 0, 1) for dx in (-1, 0, 1)]
TAPS5 = [(dy, dx) for dy in (-2, -1, 0, 1, 2) for dx in (-2, -1, 0, 1, 2)]
TAPS3D = [(dy, dx) for dy in (-2, 0, 2) for dx in (-2, 0, 2)]
TAPS5D = [(dy, dx) for dy in (-4, -2, 0, 2, 4) for dx in (-4, -2, 0, 2, 4)]


def build_plan(inputs):
    g = _gate_math(inputs)
    coef = g["coef"]

    scale0 = _f32(inputs["pre0_g"]) * g["gv0"]
    bias0 = _f32(inputs["pre0_b"]) * g["gv0"]
    scale1 = _f32(inputs["pre1_g"]) * g["gv1"]
    bias1 = _f32(inputs["pre1_b"]) * g["gv1"]
    wpre0 = (_f32(inputs["pre0_w"]) * scale0[:, None]).T.copy()  # (512,128)
    wpre1 = (_f32(inputs["pre1_w"]) * scale1[:, None]).T.copy()

    state_of_edge = []
    for i in range(STEPS):
        for j in range(2 + i):
            state_of_edge.append((i, j))

    edges = []
    state_bias = np.zeros((6, C), np.float32)
    for e in range(N_EDGES):
        i, j = state_of_edge[e]
        tgt = 2 + i
        ops = {
            "max": float(coef[e, 1]),
            "avg": float(coef[e, 2]),
            "skip": float(coef[e, 3]),
        }
        for k, nm, taps in ((4, "sep3", TAPS3), (5, "sep5", TAPS5)):
            c = float(coef[e, k])
            if c == 0.0:
                ops[nm] = None
                continue
            gate = g["gates"][e, k]
            mk = g["mk"][e, k]
            mw = g["mw"][e, k]
            s1 = _f32(inputs[f"{nm}_g1"][e]) * gate * mk
            bb1 = _f32(inputs[f"{nm}_b1"][e]) * gate * mk
            s2 = c * _f32(inputs[f"{nm}_g2"][e]) * gate * mw
            bb2 = c * _f32(inputs[f"{nm}_b2"][e]) * gate * mw
            state_bias[tgt] += bb2
            if not s2.any() or not (s1.any() or bb1.any()):
                ops[nm] = None
                continue
            ops[nm] = dict(
                taps=taps,
                layers=[
                    dict(
                        dw=_f32(inputs[f"{nm}_dw1"][e]),
                        pw=_f32(inputs[f"{nm}_pw1"][e]),
                        scale=s1,
                    ),
                    dict(
                        dw=_f32(inputs[f"{nm}_dw2"][e]),
                        pw=_f32(inputs[f"{nm}_pw2"][e]),
                        scale=s2,
                    ),
                ],
                bias1=bb1,
            )
        for k, nm, taps in ((6, "dil3", TAPS3D), (7, "dil5", TAPS5D)):
            c = float(coef[e, k])
            if c == 0.0:
                ops[nm] = None
                continue
            gate = g["gates"][e, k]
            mk = g["mk"][e, k]
            s = c * _f32(inputs[f"{nm}_g"][e]) * gate * mk
            state_bias[tgt] += c * _f32(inputs[f"{nm}_b"][e]) * gate * mk
            if not s.any():
                ops[nm] = None
                continue
            ops[nm] = dict(
                taps=taps,
                layers=[
                    dict(
                        dw=_f32(inputs[f"{nm}_dw"][e]),
                        pw=_f32(inputs[f"{nm}_pw"][e]),
                        scale=s,
                    )
                ],
            )
        if ops["dil3"] is not None and ops["dil5"] is not None:
            # merge dil3 into dil5 (same taps grid superset, same target acc)
            lay5 = ops["dil5"]["layers"][0]
            lay5["merge"] = ops["dil3"]["layers"][0]
            lay5["merge_taps"] = TAPS3D
            ops["dil3"] = None
        edges.append(dict(e=e, step=i, src=j, tgt=tgt, ops=ops))

    cnt1 = np.full(HH, 3.0, np.float32)
    cnt1[0] = cnt1[-1] = 2.0
    cnt = np.float32(1.0) / np.outer(cnt1, cnt1).astype(np.float32)
    rcnt = np.broadcast_to(cnt.reshape(1, PIX), (C, PIX)).copy()

    plan = dict(
        edges=edges,
        wpre0=wpre0,
        wpre1=wpre1,
        bias0=bias0,
        bias1=bias1,
        state_bias=state_bias,
        rcnt=rcnt,
    )
    _fuse_weights_dr(plan)
    s0 = _f32(inputs["s0"]).reshape(B, C_PREV, PIX)
    s1 = _f32(inputs["s1"]).reshape(B, C_PREV, PIX)
    calib_ids = [0, B // 2]
    _apply_scales(plan, [(s0[b], s1[b]) for b in calib_ids])
    build_wall(plan)
    plan["wall"] = plan["wall8"]  # back-compat alias
    return plan


def layer_tap_mats(lay, taps):
    """Per-tap fused (C_in, C_out) matrices for one conv layer."""
    T = len(taps)
    dwf = lay["dw"].reshape(C, T)
    pws = (lay["pw"] * lay["scale"][:, None]).T  # (Cin, O)
    mats = {t: dwf[:, ti : ti + 1] * pws for ti, t in enumerate(taps)}
    if "merge" in lay:
        mlay = lay["merge"]
        mtaps = lay["merge_taps"]
        mdw = mlay["dw"].reshape(C, len(mtaps))
        mpws = (mlay["pw"] * mlay["scale"][:, None]).T
        for mi, t in enumerate(mtaps):
            mats[t] = mats[t] + mdw[:, mi : mi + 1] * mpws
    return mats


# ---------------------------------------------------------------------------
# Scale calibration (host numpy forward in f32)
# ---------------------------------------------------------------------------


def _pad_img(x, pad, fill=0.0):
    out = np.full((C, HH + 2 * pad, WW + 2 * pad), fill, np.float32)
    out[:, pad : pad + HH, pad : pad + WW] = x
    return out


def _win(xpad, pad, dy, dx):
    return xpad[:, pad + dy : pad + dy + HH, pad + dx : pad + dx + WW].reshape(C, PIX)


def _forward_f32(plan, s0, s1, state_absmax=None, mid_absmax=None):
    """f32 forward on one image; optionally record absmax stats."""
    states = []
    for s, w, bia in (
        (s0, plan["wpre0"], plan["bias0"]),
        (s1, plan["wpre1"], plan["bias1"]),
    ):
        r = np.maximum(s, 0.0)
        h = (w.T @ r + bia[:, None]).astype(np.float32)
        states.append(h)

    for i in range(STEPS):
        tgt = 2 + i
        acc = np.zeros((C, PIX), np.float32)
        acc += plan["state_bias"][tgt][:, None]
        for ed in plan["edges"]:
            if ed["step"] != i:
                continue
            x = states[ed["src"]].reshape(C, HH, WW)
            ops = ed["ops"]
            if ops["max"] != 0.0:
                xm = _pad_img(x, 1, -np.inf)
                m = np.full((C, HH, WW), -np.inf, np.float32)
                for dy in (-1, 0, 1):
                    for dx in (-1, 0, 1):
                        m = np.maximum(
                            m, xm[:, 1 + dy : 1 + dy + HH, 1 + dx : 1 + dx + WW]
                        )
                acc += ops["max"] * m.reshape(C, PIX)
            if ops["avg"] != 0.0:
                xa = _pad_img(x, 1, 0.0)
                ssum = np.zeros((C, HH, WW), np.float32)
                for dy in (-1, 0, 1):
                    for dx in (-1, 0, 1):
                        ssum += xa[:, 1 + dy : 1 + dy + HH, 1 + dx : 1 + dx + WW]
                acc += ops["avg"] * (ssum.reshape(C, PIX) * plan["rcnt"])
            if ops["skip"] != 0.0:
                acc += ops["skip"] * x.reshape(C, PIX)
            rp = _pad_img(np.maximum(x, 0.0), RPAD_P)
            for nm in ("sep3", "sep5"):
                op = ops[nm]
                if op is None:
                    continue
                mats = layer_tap_mats(op["layers"][0], op["taps"])
                mid = np.zeros((C, PIX), np.float32)
                for (dy, dx), M in mats.items():
                    mid += M.T @ _win(rp, RPAD_P, dy, dx)
                mid = np.maximum(mid + op["bias1"][:, None], 0.0)
                if mid_absmax is not None:
                    k = (ed["e"], nm)
                    mid_absmax[k] = max(mid_absmax.get(k, 0.0), float(np.abs(mid).max()))
                mp = _pad_img(mid.reshape(C, HH, WW), MPAD_P)
                mats2 = layer_tap_mats(op["layers"][1], op["taps"])
                for (dy, dx), M in mats2.items():
                    acc += M.T @ _win(mp, MPAD_P, dy, dx)
            for nm in ("dil3", "dil5"):
                op = ops[nm]
                if op is None:
                    continue
                mats = layer_tap_mats(op["layers"][0], op["taps"])
                for (dy, dx), M in mats.items():
                    acc += M.T @ _win(rp, RPAD_P, dy, dx)
        states.append(acc)

    if state_absmax is not None:
        for si, st in enumerate(states):
            state_absmax[si] = max(state_absmax[si], float(np.abs(st).max()))
    return states


def _pow2(v):
    return float(2.0 ** np.floor(np.log2(max(v, 1e-30))))


def _apply_scales(plan, calib_pairs):
    state_absmax = [1e-6] * 6
    mid_absmax = {}
    for s0, s1 in calib_pairs:
        _forward_f32(plan, s0, s1, state_absmax, mid_absmax)

    SX = [_pow2(ACT_TARGET / v) for v in state_absmax]
    SM = {}
    for ed in plan["edges"]:
        for nm in ("sep3", "sep5"):
            op = ed["ops"][nm]
            if op is None:
                continue
            k = (ed["e"], nm)
            sm = _pow2(ACT_TARGET / max(mid_absmax.get(k, 1e-6), 1e-6))
            # clamp so L1 weights (scaled by SM/SX) stay in range
            mats = layer_tap_mats(op["layers"][0], op["taps"])
            m1 = max(float(np.abs(M).max()) for M in mats.values())
            sx = SX[ed["src"]]
            if m1 > 0:
                sm = min(sm, _pow2(2.0 * W_TARGET * sx / m1))
            SM[k] = sm
    SACC = {}
    for i in range(STEPS):
        cap = 1e30
        for ed in plan["edges"]:
            if ed["step"] != i:
                continue
            for nm in ("sep3", "sep5"):
                op = ed["ops"][nm]
                if op is None:
                    continue
                mats = layer_tap_mats(op["layers"][1], op["taps"])
                m2 = max(float(np.abs(M).max()) for M in mats.values())
                if m2 > 0:
                    cap = min(cap, W_TARGET * SM[(ed["e"], nm)] / m2)
            for nm in ("dil3", "dil5"):
                op = ed["ops"][nm]
                if op is None:
                    continue
                mats = layer_tap_mats(op["layers"][0], op["taps"])
                m = max(float(np.abs(M).max()) for M in mats.values())
                if m > 0:
                    cap = min(cap, W_TARGET * SX[ed["src"]] / m)
        SACC[i] = _pow2(cap) if cap < 1e29 else 1.0
    plan["SX"] = SX
    plan["SM"] = SM
    plan["SACC"] = SACC


# ---------------------------------------------------------------------------
# DoubleRow tap pairing + fp8 wall
# ---------------------------------------------------------------------------


R_SLAB = RPAD_W * RPAD_W  # 1600 elements per rpad slab
M_SLAB = MPAD_W * MPAD_W  # 1296 elements per mpad slab
N_MPAD = 4  # mpad rotation slots (+1 zero slab in the super-tile)


def pair_taps(taps):
    """Pair taps under the hw constraint: DR k-tile stride multiple of 4.

    With pad widths % 4 == 0 this means dx difference must be 0 mod 4.
    Returns (pairs, leftover_singles).
    """
    from collections import defaultdict

    groups = defaultdict(list)
    for t in sorted(taps, key=lambda t: (t[1], t[0])):
        groups[t[1] % 4].append(t)
    pairs, singles = [], []
    for k in sorted(groups):
        ts_ = groups[k]
        for i in range(0, len(ts_) - 1, 2):
            pairs.append((ts_[i], ts_[i + 1]))
        if len(ts_) % 2:
            singles.append(ts_[-1])
    return pairs, singles


def _abs_base(kind, slab, tap, pad, W):
    slab_sz = R_SLAB if kind == "r" else M_SLAB
    return slab * slab_sz + (pad + tap[0]) * W + (pad + tap[1])


def _zero_base(plan, kind, ref_base):
    """A window base inside the all-zero slab, congruent to ref_base mod 4."""
    if kind == "r":
        return plan["zero_rslab"] * R_SLAB + RPAD_P * RPAD_W + (ref_base % 4)
    return N_MPAD * M_SLAB + MPAD_P * MPAD_W + (ref_base % 4)


def _fuse_weights_dr(plan):
    """Per-layer DR pairs; leftover singles cross-paired within each step
    (acc-feeding layers, same super-tile, same dx%4 class); remaining singles
    pair against the zero slab (their second 128 lhsT columns are junk that
    multiplies zeros)."""
    conv_srcs = sorted(
        {
            ed["src"]
            for ed in plan["edges"]
            if any(ed["ops"][nm] is not None for nm in ("sep3", "sep5", "dil3", "dil5"))
        }
    )
    plan["conv_srcs"] = conv_srcs
    slab_of_src = {s: i for i, s in enumerate(conv_srcs)}
    plan["slab_of_src"] = slab_of_src
    plan["zero_rslab"] = len(conv_srcs)

    rot = 0
    for ed in plan["edges"]:
        for nm in ("sep3", "sep5"):
            op = ed["ops"][nm]
            if op is not None:
                op["mpad_slot"] = rot % N_MPAD
                rot += 1

    step_singles = {i: [] for i in range(STEPS)}
    for ed in plan["edges"]:
        for nm in ("sep3", "sep5", "dil3", "dil5"):
            op = ed["ops"][nm]
            if op is None:
                continue
            op["emit"] = []
            nlay = len(op["layers"])
            for li in range(nlay):
                if li == 0:
                    kind, pad, W = "r", RPAD_P, RPAD_W
                    slab = slab_of_src[ed["src"]]
                else:
                    kind, pad, W = "m", MPAD_P, MPAD_W
                    slab = op["mpad_slot"]
                pairs, singles = pair_taps(op["taps"])
                ent = dict(
                    kind=kind, pad=pad, W=W, slab=slab,
                    e=ed["e"], nm=nm, li=li, units=[], zsingles=[],
                )
                for ta, tb in pairs:
                    b1 = _abs_base(kind, slab, ta, pad, W)
                    b2 = _abs_base(kind, slab, tb, pad, W)
                    if b2 < b1:
                        ta, tb, b1, b2 = tb, ta, b2, b1
                    assert (b2 - b1) % 4 == 0
                    ent["units"].append(
                        dict(kind=kind, base=b1, ks=b2 - b1,
                             specs=((ent, ta), (ent, tb)))
                    )
                if li == nlay - 1:
                    # acc-feeding layer (dil li==0, sep li==1): cross-pairable
                    for t in singles:
                        step_singles[ed["step"]].append((kind, t[1] % 4, t, ent))
                else:
                    ent["zsingles"].extend(singles)
                op["emit"].append(ent)

    plan["cross_units"] = {i: [] for i in range(STEPS)}
    from collections import defaultdict

    for i in range(STEPS):
        g = defaultdict(list)
        for kind, cls, t, ent in step_singles[i]:
            b = _abs_base(kind, ent["slab"], t, ent["pad"], ent["W"])
            g[(kind, cls)].append((b, t, ent))
        for key in sorted(g):
            lst = sorted(g[key], key=lambda x: x[0])
            j = 0
            while j + 1 < len(lst):
                b1, t1, e1 = lst[j]
                b2, t2, e2 = lst[j + 1]
                if b2 == b1:  # identical window: cannot pair, zero-slab one
                    e1["zsingles"].append(t1)
                    j += 1
                    continue
                assert (b2 - b1) % 4 == 0
                plan["cross_units"][i].append(
                    dict(kind=key[0], base=b1, ks=b2 - b1,
                         specs=((e1, t1), (e2, t2)))
                )
                j += 2
            if j < len(lst):
                b1, t1, e1 = lst[j]
                e1["zsingles"].append(t1)


def _layer_sw(plan, ent):
    SX, SM, SACC = plan["SX"], plan["SM"], plan["SACC"]
    e, nm, li = ent["e"], ent["nm"], ent["li"]
    ed = plan["edges"][e]
    if nm in ("sep3", "sep5"):
        return (
            SM[(e, nm)] / SX[ed["src"]] if li == 0 else SACC[ed["step"]] / SM[(e, nm)]
        )
    return SACC[ed["step"]] / SX[ed["src"]]


def build_wall(plan):
    """Quantize all unit matrices into the fp8 wall (column-addressed)."""
    fp8 = _fp8_dtype()
    mats_cache = {}

    def get_mat(ent, tap):
        key = (ent["e"], ent["nm"], ent["li"])
        if key not in mats_cache:
            op = plan["edges"][ent["e"]]["ops"][ent["nm"]]
            lay = op["layers"][ent["li"]]
            s_w = _layer_sw(plan, ent)
            mats_cache[key] = {
                t: M * s_w for t, M in layer_tap_mats(lay, op["taps"]).items()
            }
        return mats_cache[key][tap]

    blocks = []
    col = 0

    def chunk_for(ent_units, zs_specs):
        """zsingles first (128 cols), then pairs (256 cols)."""
        nonlocal col
        start = col
        for ent, t in zs_specs:
            blocks.append(get_mat(ent, t))
            col += 128
        for u in ent_units:
            (ea, ta), (eb, tb) = u["specs"]
            blocks.append(get_mat(ea, ta))
            blocks.append(get_mat(eb, tb))
            u["col"] = col - start
            col += 256
        if not ent_units:  # junk-read tail for the last zsingle
            blocks.append(np.zeros((C, 128), np.float32))
            col += 128
        return start

    for ed in plan["edges"]:
        for nm in ("sep3", "sep5", "dil3", "dil5"):
            op = ed["ops"][nm]
            if op is None:
                continue
            for ent in op["emit"]:
                zs = [(ent, t) for t in ent["zsingles"]]
                ent["wall_col"] = chunk_for(ent["units"], zs)
                # per-zsingle units (col offsets relative to chunk)
                ent["zunits"] = []
                for zi, t in enumerate(ent["zsingles"]):
                    b = _abs_base(ent["kind"], ent["slab"], t, ent["pad"], ent["W"])
                    zb = _zero_base(plan, ent["kind"], b)
                    assert (zb - b) % 4 == 0 and zb > b
                    ent["zunits"].append(
                        dict(kind=ent["kind"], base=b, ks=zb - b, col=zi * 128)
                    )
                ent["chunk_cols"] = col - ent["wall_col"]

    for i in range(STEPS):
        cu = plan["cross_units"][i]
        if not cu:
            continue
        start = col
        for u in cu:
            (ea, ta), (eb, tb) = u["specs"]
            blocks.append(get_mat(ea, ta))
            blocks.append(get_mat(eb, tb))
            u["col"] = col - start
            col += 256
        plan.setdefault("cross_wall", {})[i] = (start, col - start)

    wall_f32 = (
        np.concatenate(blocks, axis=1) if blocks else np.zeros((C, 256), np.float32)
    )
    amax = float(np.abs(wall_f32).max())
    assert amax < 239.0, f"fp8 weight overflow: {amax}"
    plan["wall8"] = wall_f32.astype(fp8)
    plan["n_wall_cols"] = max(col, 128)
    plan["n_units"] = sum(
        len(ent["units"]) + len(ent["zsingles"])
        for ed in plan["edges"]
        for nm in ("sep3", "sep5", "dil3", "dil5")
        if ed["ops"][nm] is not None
        for ent in ed["ops"][nm]["emit"]
    ) + sum(len(v) for v in plan["cross_units"].values())


# ---------------------------------------------------------------------------
# Bass device program
# ---------------------------------------------------------------------------


def build_device_program(plan):
    from contextlib import ExitStack

    import concourse.bacc as bacc
    import concourse.mybir as mybir
    import concourse.tile as tile
    from concourse.ap import AP

    F32 = mybir.dt.float32
    F32R = mybir.dt.float32r
    BF16 = mybir.dt.bfloat16
    FP8 = mybir.dt.float8e4
    AO = mybir.AluOpType
    AF = mybir.ActivationFunctionType
    DRMODE = mybir.MatmulPerfMode.DoubleRow
    AOm, AOa = AO.mult, AO.add

    SX, SM, SACC = plan["SX"], plan["SM"], plan["SACC"]

    nc = bacc.Bacc("TRN2", target_bir_lowering=False, debug=False)
    d_st01 = nc.dram_tensor("st01", [2, 128, PIX], BF16, kind="ExternalInput").ap()
    d_rp01 = nc.dram_tensor("rp01", [2, 128, PIX], FP8, kind="ExternalInput").ap()
    d_wall = nc.dram_tensor("wall", [128, plan["n_wall_cols"]], FP8, kind="ExternalInput").ap()
    d_btab = nc.dram_tensor("btab", [128, 64], F32, kind="ExternalInput").ap()
    d_rcnt = nc.dram_tensor("rcnt", [128, PIX], BF16, kind="ExternalInput").ap()
    d_out = nc.dram_tensor("out", [4, 128, PIX], F32, kind="ExternalOutput").ap()

    # bias table columns
    bias_cols = {}
    next_bias = 6
    for ed in plan["edges"]:
        for nm in ("sep3", "sep5"):
            if ed["ops"][nm] is not None:
                bias_cols[(ed["e"], nm)] = next_bias
                next_bias += 1
    assert next_bias <= 64

    # NOTE: scalar_tensor_tensor is NOT supported on the Pool/gpsimd engine
    # (walrus ISA check rejects it), so extras stay on DVE by default.
    gp_extra = int(os.environ.get("KERNEL_GP_EXTRA", "0"))

    with tile.TileContext(nc) as tc, ExitStack() as ctx:
        const = ctx.enter_context(tc.tile_pool(name="const", bufs=1))
        stp = ctx.enter_context(tc.tile_pool(name="stp", bufs=1))
        stb = ctx.enter_context(tc.tile_pool(name="stb", bufs=1))
        poolp = ctx.enter_context(tc.tile_pool(name="poolp", bufs=1))
        rpadp = ctx.enter_context(tc.tile_pool(name="rpadp", bufs=1))
        mpadp = ctx.enter_context(tc.tile_pool(name="mpadp", bufs=1))
        extrap = ctx.enter_context(tc.tile_pool(name="extrap", bufs=3))
        psum = ctx.enter_context(tc.tile_pool(name="psum", bufs=2, space="PSUM"))

        btab = const.tile([128, 64], F32, tag="btab", name="btab")
        nc.gpsimd.dma_start(btab[:], d_btab)
        rcnt = const.tile([128, PIX], BF16, tag="rcnt", name="rcnt")
        nc.gpsimd.dma_start(rcnt[:], d_rcnt)

        def bias_ap(col):
            return btab[:, col : col + 1]

        # super-tiles: per-source rpad slabs + zero slab; mpad rotation slabs +
        # zero slab. Memset upfront per slab (pad rings + zero slabs stay 0).
        conv_srcs = plan["conv_srcs"]
        slab_of_src = plan["slab_of_src"]
        rsuper = rpadp.tile(
            [128, (len(conv_srcs) + 1) * R_SLAB], FP8, tag="rsuper", name="rsuper"
        )
        msuper = mpadp.tile(
            [128, (N_MPAD + 1) * M_SLAB], FP8, tag="msuper", name="msuper"
        )
        rv_ = rsuper[:]
        mv_ = msuper[:]

        def ring_memset(v, slab_off, W, pad):
            m = nc.gpsimd.memset
            def ap2(off, rows, cols):
                return AP(v.tensor, v.offset + slab_off + off, [[v.ap[0][0], 128], [W, rows], [1, cols]])
            m(ap2(0, pad, W), 0.0)
            m(ap2((W - pad) * W, pad, W), 0.0)
            m(ap2(pad * W, W - 2 * pad, pad), 0.0)
            m(ap2(pad * W + W - pad, W - 2 * pad, pad), 0.0)

        headv = os.environ.get("KERNEL_HEADV", "2")
        nZr = len(conv_srcs)
        if headv == "2":
            for sl in range(min(2, nZr)):
                ring_memset(rv_, sl * R_SLAB, RPAD_W, RPAD_P)
            nc.gpsimd.memset(rsuper[:, nZr * R_SLAB : (nZr + 1) * R_SLAB], 0.0)
            nc.gpsimd.memset(msuper[:, N_MPAD * M_SLAB : (N_MPAD + 1) * M_SLAB], 0.0)
            for sl in range(2, nZr):
                ring_memset(rv_, sl * R_SLAB, RPAD_W, RPAD_P)
            for sl in range(N_MPAD):
                ring_memset(mv_, sl * M_SLAB, MPAD_W, MPAD_P)
        else:
            for sl in range(nZr + 1):
                nc.gpsimd.memset(rsuper[:, sl * R_SLAB : (sl + 1) * R_SLAB], 0.0)
            for sl in range(N_MPAD + 1):
                nc.gpsimd.memset(msuper[:, sl * M_SLAB : (sl + 1) * M_SLAB], 0.0)

        rv = rsuper[:]
        mv = msuper[:]
        sup_v = {"r": rv, "m": mv}
        sup_W = {"r": RPAD_W, "m": MPAD_W}

        dma_rr = [0]
        wq_names = os.environ.get("KERNEL_WQUEUES", "sync").split(",")
        wq_map = {"sync": nc.sync, "gpsimd": nc.gpsimd, "scalar": nc.scalar, "vector": nc.vector}
        w_queues = [wq_map[n] for n in wq_names]

        def rr_queue():
            q = w_queues[dma_rr[0] % len(w_queues)]
            dma_rr[0] += 1
            return q

        wp = ctx.enter_context(tc.tile_pool(name="wp", bufs=3))

        def dma_chunk(wall_col, cols):
            bucket = (cols + 1023) // 1024 * 1024
            wt = wp.tile(
                [128, bucket],
                FP8,
                tag=f"w{bucket}",
                name="wt",
                bufs=int(os.environ.get("KERNEL_WBUFS", "6")),
            )
            rr_queue().dma_start(wt[:, 0:cols], d_wall[:, wall_col : wall_col + cols])
            return wt

        def dma_layer(ent):
            return dma_chunk(ent["wall_col"], ent["chunk_cols"])

        # ---- states 0/1 and their fp8 rpads computed on host, streamed in
        states = []
        for si in range(2):
            stt = stp.tile([128, HH, WW], BF16, tag=f"state{si}", name=f"state{si}")
            nc.sync.dma_start(stt[:].rearrange("p a b -> p (a b)"), d_st01[si])
            states.append(stt)
        rpad_preload = {}
        for si in range(2):
            if si not in slab_of_src:
                continue
            off = slab_of_src[si] * R_SLAB + RPAD_P * RPAD_W + RPAD_P
            dst = AP(
                rv.tensor,
                rv.offset + off,
                [[rv.ap[0][0], 128], [RPAD_W, HH], [1, WW]],
            )
            nc.sync.dma_start(dst, d_rp01[si])
            rpad_preload[si] = True

        scratch = ctx.enter_context(tc.tile_pool(name="scratch", bufs=3))

        # lazy caches
        rpad_cache = {}
        st16_cache = {}
        maxp_cache = {}
        avgp_cache = {}

        def get_rpad(s):
            if s in rpad_preload:
                return True
            if s not in rpad_cache:
                off = slab_of_src[s] * R_SLAB + RPAD_P * RPAD_W + RPAD_P
                out = AP(
                    rv.tensor,
                    rv.offset + off,
                    [[rv.ap[0][0], 128], [RPAD_W, HH], [1, WW]],
                )
                nc.scalar.activation(out, states[s][:], AF.Relu, scale=float(SX[s]))
                rpad_cache[s] = True
            return rpad_cache[s]

        def get_st16(s):
            if s < 2:
                return states[s]
            if s not in st16_cache:
                t = stb.tile([128, HH, WW], BF16, tag=f"st16_{s}", name=f"st16_{s}")
                nc.scalar.activation(t[:], states[s][:], AF.Copy)
                st16_cache[s] = t
            return st16_cache[s]

        def pool_pass(x, out, tmp, op):
            tt = nc.vector.tensor_tensor
            tt(tmp[:, :, 1:31], x[:, :, 0:30], x[:, :, 1:31], op=op)
            tt(tmp[:, :, 1:31], tmp[:, :, 1:31], x[:, :, 2:32], op=op)
            tt(tmp[:, :, 0:1], x[:, :, 0:1], x[:, :, 1:2], op=op)
            tt(tmp[:, :, 31:32], x[:, :, 30:31], x[:, :, 31:32], op=op)
            tt(out[:, 1:31, :], tmp[:, 0:30, :], tmp[:, 1:31, :], op=op)
            tt(out[:, 1:31, :], out[:, 1:31, :], tmp[:, 2:32, :], op=op)
            tt(out[:, 0:1, :], tmp[:, 0:1, :], tmp[:, 1:2, :], op=op)
            tt(out[:, 31:32, :], tmp[:, 30:31, :], tmp[:, 31:32, :], op=op)

        def get_maxp(s):
            if s not in maxp_cache:
                x16 = get_st16(s)
                tmp = scratch.tile([128, HH, WW], BF16, tag="ptmp", name="ptmp", bufs=2)
                out = poolp.tile([128, HH, WW], BF16, tag=f"maxp{s}", name=f"maxp{s}")
                pool_pass(x16, out, tmp, AO.max)
                maxp_cache[s] = out
            return maxp_cache[s]

        def get_avgp(s):
            if s not in avgp_cache:
                x16 = get_st16(s)
                tmp = scratch.tile([128, HH, WW], BF16, tag="ptmp", name="ptmp", bufs=2)
                out = poolp.tile([128, HH, WW], BF16, tag=f"avgp{s}", name=f"avgp{s}")
                pool_pass(x16, out, tmp, AO.add)
                nc.vector.tensor_tensor(
                    out[:].rearrange("p a b -> p (a b)"),
                    out[:].rearrange("p a b -> p (a b)"),
                    rcnt[:],
                    op=AOm,
                )
                avgp_cache[s] = out
            return avgp_cache[s]

        extra_ctr = [0]

        def emit_extra(extra, in_ap, coef):
            eng = (
                nc.gpsimd
                if gp_extra > 0 and extra_ctr[0] % gp_extra == gp_extra - 1
                else nc.vector
            )
            eng.scalar_tensor_tensor(
                extra[:], in_ap, float(coef), extra[:], op0=AOm, op1=AOa
            )
            extra_ctr[0] += 1

        def emit_unit_list(units, wt, out_fn, h_major=False):
            """units: dicts with kind/base/ks/col; out_fn(h, lhsT, rhs).
            h_major: all h=0 matmuls first so that psum half stops earlier."""
            def one(u, h):
                v = sup_v[u["kind"]]
                W = sup_W[u["kind"]]
                lhsT = wt[:, u["col"] : u["col"] + 256].rearrange(
                    "p (two m) -> p two m", two=2
                )
                rhs = AP(
                    v.tensor,
                    v.offset + u["base"] + 16 * h * W,
                    [[v.ap[0][0], 128], [u["ks"], 2], [W, 16], [1, 32]],
                )
                out_fn(h, lhsT, rhs)

            if h_major:
                for h in range(2):
                    for u in units:
                        one(u, h)
            else:
                for u in units:
                    one(u, 0)
                    one(u, 1)

        def emit_units(ent, wt, out_fn):
            emit_unit_list(ent["zunits"], wt, out_fn)
            emit_unit_list(ent["units"], wt, out_fn)

        for i in range(STEPS):
            tgt = 2 + i
            step_edges = [ed for ed in plan["edges"] if ed["step"] == i]

            n_acc = len(plan["cross_units"][i])
            for ed in step_edges:
                for nm in ("sep3", "sep5", "dil3", "dil5"):
                    op = ed["ops"][nm]
                    if op is None:
                        continue
                    ent = op["emit"][-1]
                    n_acc += len(ent["units"]) + len(ent["zsingles"])

            acc = psum.tile([128, PIX], F32, tag="acc", name="acc") if n_acc else None
            extra = extrap.tile([128, PIX], F32, tag="extra", name="extra")
            # extra starts as the hoisted per-channel bias
            nc.scalar.activation(
                extra[:].rearrange("p (a b) -> p a b", a=HH),
                rcnt[:].rearrange("p (a b) -> p a b", a=HH),
                AF.Identity,
                bias=bias_ap(2 + i),
                scale=0.0,
            )

            acc_idx = [0, 0]

            def acc_mm(h, lhsT, rhs):
                nc.tensor.matmul(
                    acc[:, 512 * h : 512 * (h + 1)],
                    lhsT,
                    rhs,
                    start=(acc_idx[h] == 0),
                    stop=(acc_idx[h] == n_acc - 1),
                    perf_mode=DRMODE,
                )
                acc_idx[h] += 1

            # pass 1: dil units + sep L1 -> mpad (ACT); sep L2 deferred to
            # pass 2 so the tensor engine never stalls on an mpad write.
            l2_queue = []
            for ed in step_edges:
                s = ed["src"]
                ops = ed["ops"]
                x = states[s]

                live = [
                    nm for nm in ("sep3", "sep5", "dil3", "dil5") if ops[nm] is not None
                ]
                if live:
                    get_rpad(s)
                    # DMA issue order (tuned): KERNEL_DMAORD picks the variant
                    tiles = {nm: [None] * len(ops[nm]["emit"]) for nm in live}
                    order_v = os.environ.get("KERNEL_DMAORD", "c")
                    if order_v == "a":  # original: sep3[L1,L2], sep5[L1,L2], dils
                        seq = [(nm, li) for nm in live for li in range(len(ops[nm]["emit"]))]
                    elif order_v == "b":  # dils, sep L1s, sep L2s
                        seq = (
                            [(nm, 0) for nm in ("dil3", "dil5") if nm in live]
                            + [(nm, 0) for nm in ("sep3", "sep5") if nm in live]
                            + [(nm, 1) for nm in ("sep3", "sep5") if nm in live]
                        )
                    else:  # "c": dils, then sep3[L1,L2], sep5[L1,L2]
                        seq = [(nm, 0) for nm in ("dil3", "dil5") if nm in live] + [
                            (nm, li)
                            for nm in ("sep3", "sep5")
                            if nm in live
                            for li in range(2)
                        ]
                    for nm, li in seq:
                        tiles[nm][li] = dma_layer(ops[nm]["emit"][li])
                    for nm in ("dil3", "dil5"):
                        if nm in live:
                            emit_units(ops[nm]["emit"][0], tiles[nm][0], acc_mm)
                    for nm in ("sep3", "sep5"):
                        if nm not in live:
                            continue
                        op = ops[nm]
                        ent1, ent2 = op["emit"]
                        mid = [
                            psum.tile([128, 512], F32, tag="mid", name="mid", bufs=4)
                            for _ in range(2)
                        ]
                        n1 = len(ent1["units"]) + len(ent1["zsingles"])
                        mm_i = [0, 0]

                        def mid_mm(h, lhsT, rhs, mid=mid, mm_i=mm_i, n1=n1):
                            nc.tensor.matmul(
                                mid[h][:],
                                lhsT,
                                rhs,
                                start=(mm_i[h] == 0),
                                stop=(mm_i[h] == n1 - 1),
                                perf_mode=DRMODE,
                            )
                            mm_i[h] += 1

                        emit_units(ent1, tiles[nm][0], mid_mm)
                        slot = op["mpad_slot"]
                        for h in range(2):
                            out = AP(
                                mv.tensor,
                                mv.offset
                                + slot * M_SLAB
                                + (MPAD_P + 16 * h) * MPAD_W
                                + MPAD_P,
                                [[mv.ap[0][0], 128], [MPAD_W, 16], [1, WW]],
                            )
                            nc.scalar.activation(
                                out,
                                mid[h][:].rearrange("p (a b) -> p a b", a=16),
                                AF.Relu,
                                bias=bias_ap(bias_cols[(ed["e"], nm)]),
                            )
                        l2_queue.append((ent2, tiles[nm][1]))

                # pool/skip contributions
                if ops["max"] != 0.0:
                    mp = get_maxp(s)
                    emit_extra(extra, mp[:].rearrange("p a b -> p (a b)"), ops["max"])
                if ops["avg"] != 0.0:
                    ap_ = get_avgp(s)
                    emit_extra(extra, ap_[:].rearrange("p a b -> p (a b)"), ops["avg"])
                if ops["skip"] != 0.0:
                    emit_extra(extra, x[:].rearrange("p a b -> p (a b)"), ops["skip"])

            # pass 2: sep L2 units (mpads computed during pass 1)
            for ent2, wt2 in l2_queue:
                emit_units(ent2, wt2, acc_mm)

            # pass 3: cross-paired leftover singles (rpad and mpad mixed)
            cu = plan["cross_units"][i]
            if cu:
                cstart, ccols = plan["cross_wall"][i]
                cwt = dma_chunk(cstart, ccols)
                emit_unit_list(cu, cwt, acc_mm)

            assert acc_idx[0] == n_acc and acc_idx[1] == n_acc, (acc_idx, n_acc)

            stt = stp.tile([128, HH, WW], F32, tag=f"state{tgt}", name=f"state{tgt}")
            sf = stt[:].rearrange("p a b -> p (a b)")
            inv = 1.0 / SACC[i]
            if acc is not None:
                for h in range(2):
                    nc.vector.scalar_tensor_tensor(
                        sf[:, 512 * h : 512 * (h + 1)],
                        acc[:, 512 * h : 512 * (h + 1)],
                        inv,
                        extra[:, 512 * h : 512 * (h + 1)],
                        op0=AOm,
                        op1=AOa,
                    )
                    nc.sync.dma_start(
                        d_out[i][:, 512 * h : 512 * (h + 1)],
                        sf[:, 512 * h : 512 * (h + 1)],
                    )
            else:
                nc.vector.tensor_scalar(sf, extra[:], 0.0, None, op0=AOa)
                for h in range(2):
                    nc.sync.dma_start(
                        d_out[i][:, 512 * h : 512 * (h + 1)],
                        sf[:, 512 * h : 512 * (h + 1)],
                    )
            states.append(stt)

    nc.compile()
    return nc


def _make_btab(plan):
    btab = np.zeros((128, 64), np.float32)
    btab[:, 0] = plan["bias0"]
    btab[:, 1] = plan["bias1"]
    for i in range(4):
        btab[:, 2 + i] = plan["state_bias"][2 + i]
    col = 6
    for ed in plan["edges"]:
        for nm in ("sep3", "sep5"):
            if ed["ops"][nm] is not None:
                # mpad bias: SM * bias1 (mpad holds SM-scaled activations)
                btab[:, col] = plan["SM"][(ed["e"], nm)] * ed["ops"][nm]["bias1"]
                col += 1
    return btab


def make_in_maps(plan, inputs):
    import ml_dtypes

    btab = _make_btab(plan)
    s0 = _f32(inputs["s0"]).reshape(B, C_PREV, PIX)
    s1 = _f32(inputs["s1"]).reshape(B, C_PREV, PIX)
    fp8 = _fp8_dtype()
    base = {
        "wall": np.ascontiguousarray(plan["wall8"]),
        "btab": btab,
        "rcnt": plan["rcnt"].astype(ml_dtypes.bfloat16),
    }
    maps = []
    for b in range(B):
        # host preprocess: states 0/1 = W^T relu(s) + bias (f32), then bf16
        st01 = np.empty((2, 128, PIX), np.float32)
        for si, (s, w, bia) in enumerate(
            ((s0[b], plan["wpre0"], plan["bias0"]), (s1[b], plan["wpre1"], plan["bias1"]))
        ):
            st01[si] = w.T @ np.maximum(s, 0.0) + bia[:, None]
        rp01 = np.stack(
            [
                (np.maximum(st01[si], 0.0) * plan["SX"][si]).astype(fp8)
                for si in range(2)
            ]
        )
        maps.append(
            {
                **base,
                "st01": st01.astype(ml_dtypes.bfloat16),
                "rp01": rp01,
            }
        )
    return maps


def kernel(**inputs):
    plan = build_plan(inputs)
    s0 = _f32(inputs["s0"]).reshape(B, C_PREV, PIX)
    s1 = _f32(inputs["s1"]).reshape(B, C_PREV, PIX)

    if os.environ.get("KERNEL_NUMPY") == "1":
        outs = []
        for b in range(B):
            sts = _forward_f32(plan, s0[b], s1[b])
            outs.append(np.stack(sts[2:]).reshape(4 * C, HH, WW))
        return np.stack(outs).astype(np.float32)

    from concourse.bass_utils import run_bass_kernel_spmd

    nc = build_device_program(plan)
    in_maps = make_in_maps(plan, inputs)
    res = run_bass_kernel_spmd(nc, in_maps, core_ids=list(range(N_CORES)))
    out = np.stack([res.results[b]["out"].reshape(4 * C, HH, WW) for b in range(B)])
    return out.astype(np.float32)
